# revision 27
# baseline (speedup 1.0000x reference)
"""Distributed Trainium2 kernel for AlternateWeaveGather (segment_reduce).

Reference computation:
    h = x @ W.T + b                      # [N, 512] linear
    out = segment_mean(h, batch, 256)    # [256, 512]

The linear commutes with the segment sum, so each core only segment-
reduces its row shard of x (one-hot matmuls on the TensorEngine) and the
tiny 512x512 linear runs once per owned segment block at the end:
    out[s] = (segsum_x[s] @ W.T) * inv[s] + b * (cnt[s] > 0)
with inv/cnt host-derived from batch (metadata, like the index tensors).

Sharding: batch is sorted, so instead of equal row counts each core
takes ALL rows of its 32 owned segments (a contiguous ~16.4K-row slice,
host-computed via searchsorted), padded to a fixed 16640 rows (16 full
supertiles + one 2-plane tail) so the SPMD program is shape-uniform.
Padding rows carry a dummy window id (63) that the final selection
matmul discards. Every core
computes its owned output block fully locally: no collective, no
cross-core traffic at all, at the cost of +6% HBM reads. (Overlapped
ncfw collectives were tried and crawl unpredictably - 90-160us - under
full DMA load.)

W/b replicated; host concatenates the 8x[32, 512] outputs.
"""

import numpy as np

import concourse.bacc as bacc
import concourse.mybir as mybir
import concourse.tile as tile
from concourse.bass_utils import run_bass_kernel_spmd

N_CORES = 8
N_ROWS = 131072
D = 512
N_SEG = 256
SEG_PER_CORE = N_SEG // N_CORES  # 32
K = 8                            # row planes per supertile
SUP = 128 * K                    # 1024 rows per supertile
N_SUP = 16                       # full supertiles per core
KT = 2                           # planes in the short tail supertile
RPC = SUP * N_SUP + 128 * KT     # 16640 rows (max owned shard is ~16.6K)
N_PLANES = K * N_SUP + KT        # 130
W_WIN = 64                       # one-hot window: rel ids 0..31 + dummy 63
DUMMY = W_WIN - 1

F32 = mybir.dt.float32
BF16 = mybir.dt.bfloat16


def build_nc():
    nc = bacc.Bacc("TRN2", target_bir_lowering=False, debug=False,
                   num_devices=N_CORES)
    x = nc.dram_tensor("x", [RPC, D], F32, kind="ExternalInput")
    # batchp[p, t*K + k] = batch_rel[SUP*t + K*p + k]  (tail: col 128+k)
    batchp = nc.dram_tensor("batchp", [128, N_PLANES], F32,
                            kind="ExternalInput")
    iota = nc.dram_tensor("iota", [128, W_WIN], BF16, kind="ExternalInput")
    sell = nc.dram_tensor("sell", [W_WIN, SEG_PER_CORE], BF16,
                          kind="ExternalInput")
    wt = nc.dram_tensor("wt", [D, D], BF16, kind="ExternalInput")
    bindb = nc.dram_tensor("bindb", [SEG_PER_CORE, D], F32,
                           kind="ExternalInput")
    out = nc.dram_tensor("out", [SEG_PER_CORE, D], F32, kind="ExternalOutput")

    # [N_SUP, 128, K, 512]; per (t, p) the (K, 512) block is 16KB contiguous
    x_r = x.ap()[0:SUP * N_SUP, :].rearrange("(t p k) d -> t p k d",
                                             p=128, k=K)
    x_tail = x.ap()[SUP * N_SUP:RPC, :].rearrange("(p k) d -> p k d", k=KT)

    with tile.TileContext(nc) as tc:
        with tc.tile_pool(name="const", bufs=1) as const:
            iota_sb = const.tile([128, W_WIN], BF16, name="iota_sb")
            batch_sb = const.tile([128, N_PLANES], F32, name="batch_sb")
            sell_sb = const.tile([W_WIN, SEG_PER_CORE], BF16,
                                 name="sell_sb")
            wt_sb = const.tile([128, 4 * D], BF16, name="wt_sb")
            bind_sb = const.tile([SEG_PER_CORE, D], F32, name="bind_sb")
            w_sb = const.tile([W_WIN, D], BF16, name="w_sb")
            lhsT = const.tile([128, 4 * SEG_PER_CORE], BF16, name="lhsT")
            res = const.tile([SEG_PER_CORE, D], F32, name="res")

            # small inputs via SWDGE so the HWDGE x queues carry only x;
            # they land ~12us in, before the first tile's matmuls need them
            nc.gpsimd.dma_start(out=iota_sb[:, :], in_=iota[:, :])
            nc.gpsimd.dma_start(out=batch_sb[:, :], in_=batchp[:, :])
            nc.gpsimd.dma_start(out=sell_sb[:, :], in_=sell[:, :])
            for i in range(4):
                nc.gpsimd.dma_start(out=wt_sb[:, i * D:(i + 1) * D],
                                    in_=wt[i * 128:(i + 1) * 128, :])
            nc.gpsimd.dma_start(out=bind_sb[:, :], in_=bindb[:, :])

            with tc.tile_pool(name="xin", bufs=9) as xp, \
                 tc.tile_pool(name="xtail", bufs=1) as xtp, \
                 tc.tile_pool(name="ohp", bufs=16) as ohp, \
                 tc.tile_pool(name="pacc", bufs=1, space="PSUM") as pacc, \
                 tc.tile_pool(name="pepi", bufs=1, space="PSUM") as pepi:
                psW = pacc.tile([W_WIN, D], F32, name="psW")
                pt = [pepi.tile([128, SEG_PER_CORE], F32, name=f"pt{c}")
                      for c in range(4)]
                po = pepi.tile([SEG_PER_CORE, D], F32, name="po")

                def plane(oh_src_col, ps_args, xp_bf, k):
                    oh = ohp.tile([128, W_WIN], BF16, name="oh")
                    nc.vector.tensor_scalar(
                        oh[:, :], iota_sb[:, :],
                        batch_sb[:, oh_src_col:oh_src_col + 1],
                        None, mybir.AluOpType.is_equal)
                    nc.tensor.matmul(psW[:, :], oh[:, :], xp_bf[:, k, 1::2],
                                     **ps_args, skip_group_check=True)

                # every tile is partition-split across BOTH queues
                # (sync p0-63, scalar p64-127; 16KB descriptors kept):
                # tiles complete every ~5us staggered instead of in
                # pairs every ~10us, halving the compute drain
                for t in range(N_SUP):
                    xt = xp.tile([128, K, D], F32, name="xt")
                    if t == N_SUP - 1:
                        # last tile: plane-chunked for drain granularity
                        for c in range(4):
                            q = nc.sync if c % 2 == 0 else nc.scalar
                            q.dma_start(out=xt[:, 2 * c:2 * c + 2, :],
                                        in_=x_r[t][:, 2 * c:2 * c + 2, :])
                    else:
                        nc.sync.dma_start(out=xt[0:64, :, :],
                                          in_=x_r[t][0:64, :, :])
                        nc.scalar.dma_start(out=xt[64:128, :, :],
                                            in_=x_r[t][64:128, :, :])
                    # little-endian f32: the high halfword of each element
                    # IS its truncated bf16 value -> free bf16 operand
                    xt_bf = xt[:, :, :].bitcast(BF16)
                    for k in range(K):
                        plane(t * K + k,
                              dict(start=(t == 0 and k == 0), stop=False),
                              xt_bf, k)
                # short tail supertile (the shard-length slack)
                xt2 = xtp.tile([128, KT, D], F32, name="xt2")
                nc.sync.dma_start(out=xt2[:, 0:1, :], in_=x_tail[:, 0:1, :])
                nc.scalar.dma_start(out=xt2[:, 1:2, :], in_=x_tail[:, 1:2, :])
                xt2_bf = xt2[:, :, :].bitcast(BF16)
                for k in range(KT):
                    plane(N_SUP * K + k,
                          dict(start=False, stop=(k == KT - 1)), xt2_bf, k)

                # tail: window -> owned-segment selection -> tiny linear
                nc.vector.tensor_copy(w_sb[:, :], psW[:, :])
                for c in range(4):
                    nc.tensor.matmul(pt[c][:, :],
                                     w_sb[:, c * 128:(c + 1) * 128],
                                     sell_sb[:, :], start=True, stop=True,
                                     skip_group_check=True)
                for c in range(4):
                    nc.vector.tensor_copy(
                        lhsT[:, c * SEG_PER_CORE:(c + 1) * SEG_PER_CORE],
                        pt[c][:, :])
                for c in range(4):
                    nc.tensor.matmul(
                        po[:, :],
                        lhsT[:, c * SEG_PER_CORE:(c + 1) * SEG_PER_CORE],
                        wt_sb[:, c * D:(c + 1) * D],
                        start=(c == 0), stop=(c == 3),
                        skip_group_check=True)
                # res = seg_mean @ Wt + b*(cnt>0)  (inv folded into sell)
                nc.vector.tensor_tensor(res[:, :], po[:, :], bind_sb[:, :],
                                        mybir.AluOpType.add)
                nc.sync.dma_start(out=out[:, :], in_=res[:, :])
    nc.compile()
    return nc


def make_in_maps(x, W, b, batch):
    x = np.asarray(x, dtype=np.float32)
    W = np.asarray(W, dtype=np.float32)
    b = np.asarray(b, dtype=np.float32)
    batch = np.asarray(batch).astype(np.int64)
    npbf = mybir.dt.np(BF16)
    wt = np.ascontiguousarray(W.T).astype(npbf)
    iota = np.tile(np.arange(W_WIN, dtype=np.float32), (128, 1)).astype(npbf)
    counts = np.bincount(batch, minlength=N_SEG).astype(np.float64)
    inv = (1.0 / np.maximum(counts, 1.0)).astype(np.float32)
    ind = (counts > 0).astype(np.float32)

    # segment-aligned shard bounds: core j owns segments [32j, 32j+32)
    starts = [int(np.searchsorted(batch, SEG_PER_CORE * j))
              for j in range(N_CORES)] + [len(batch)]

    in_maps = []
    for j in range(N_CORES):
        lo = SEG_PER_CORE * j
        s0, s1 = starts[j], starts[j + 1]
        L = s1 - s0
        assert L <= RPC, f"core {j}: owned rows {L} exceed {RPC}"
        # row indices, padded by repeating the last row (dummy-windowed)
        ridx = np.minimum(s0 + np.arange(RPC), len(batch) - 1)
        rel = (batch[ridx] - lo).astype(np.float32)
        rel[L:] = DUMMY  # padding rows -> discarded window slot
        assert rel[:L].min() >= 0 and rel[:L].max() < SEG_PER_CORE, \
            f"core {j}: owned rel out of range"

        bp = np.empty((128, N_PLANES), np.float32)
        planes = rel[:SUP * N_SUP].reshape(N_SUP, 128, K)
        for t in range(N_SUP):
            bp[:, t * K:(t + 1) * K] = planes[t]
        bp[:, N_SUP * K:] = rel[SUP * N_SUP:].reshape(128, KT)

        sell = np.zeros((W_WIN, SEG_PER_CORE), np.float32)
        for u in range(SEG_PER_CORE):
            sell[u, u] = inv[lo + u]
        bindb = b.reshape(1, D) * ind[lo:lo + SEG_PER_CORE, None]

        in_maps.append({
            "x": np.ascontiguousarray(x[ridx]),
            "batchp": np.ascontiguousarray(bp),
            "iota": iota,
            "sell": np.ascontiguousarray(sell.astype(npbf)),
            "wt": wt,
            "bindb": np.ascontiguousarray(bindb.astype(np.float32)),
        })
    return in_maps


_NC_CACHE = {}


def kernel(x, W, b, batch, num_segments, trace=False):
    assert int(num_segments) == N_SEG
    if "nc" not in _NC_CACHE:
        _NC_CACHE["nc"] = build_nc()
    nc = _NC_CACHE["nc"]
    in_maps = make_in_maps(x, W, b, batch)
    res = run_bass_kernel_spmd(nc, in_maps, core_ids=list(range(N_CORES)),
                               trace=trace)
    full = np.concatenate([res.results[j]["out"] for j in range(N_CORES)],
                          axis=0)
    if trace:
        return full, res
    return full


# revision 28
# speedup vs baseline: 1.5550x; 1.5550x over previous
"""Distributed Trainium2 kernel for AlternateWeaveGather (segment_reduce).

Reference computation:
    h = x @ W.T + b                      # [N, 512] linear
    out = segment_mean(h, batch, 256)    # [256, 512]

The linear commutes with the segment sum, so each core only segment-
reduces its row shard of x (one-hot matmuls on the TensorEngine) and the
tiny 512x512 linear runs once per owned segment block at the end:
    out[s] = (segsum_x[s] @ W.T) * inv[s] + b * (cnt[s] > 0)
with inv/cnt host-derived from batch (metadata, like the index tensors).

Sharding: batch is sorted, so instead of equal row counts each core
takes ALL rows of its 32 owned segments (a contiguous ~16.4K-row slice,
host-computed via searchsorted), padded to a fixed 16640 rows (16 full
supertiles + one 2-plane tail) so the SPMD program is shape-uniform.
Padding rows carry a dummy window id (63) that the final selection
matmul discards. Every core
computes its owned output block fully locally: no collective, no
cross-core traffic at all, at the cost of +6% HBM reads. (Overlapped
ncfw collectives were tried and crawl unpredictably - 90-160us - under
full DMA load.)

W/b replicated; host concatenates the 8x[32, 512] outputs.
"""

import numpy as np

import concourse.bacc as bacc
import concourse.mybir as mybir
import concourse.tile as tile
from concourse.bass_utils import run_bass_kernel_spmd

N_CORES = 8
N_ROWS = 131072
D = 512
N_SEG = 256
SEG_PER_CORE = N_SEG // N_CORES  # 32
K = 8                            # row planes per supertile
SUP = 128 * K                    # 1024 rows per supertile
N_SUP = 16                       # full supertiles per core
KT = 2                           # planes in the short tail supertile
RPC = SUP * N_SUP + 128 * KT     # 16640 rows (max owned shard is ~16.6K)
N_PLANES = K * N_SUP + KT        # 130
W_WIN = 64                       # one-hot window: rel ids 0..31 + dummy 63
DUMMY = W_WIN - 1

F32 = mybir.dt.float32
BF16 = mybir.dt.bfloat16


def build_nc():
    nc = bacc.Bacc("TRN2", target_bir_lowering=False, debug=False,
                   num_devices=N_CORES)
    x = nc.dram_tensor("x", [RPC, D], F32, kind="ExternalInput")
    # batchp[p, t*K + k] = batch_rel[SUP*t + K*p + k]  (tail: col 128+k)
    batchp = nc.dram_tensor("batchp", [128, N_PLANES], F32,
                            kind="ExternalInput")
    iota = nc.dram_tensor("iota", [128, W_WIN], BF16, kind="ExternalInput")
    sell = nc.dram_tensor("sell", [W_WIN, SEG_PER_CORE], BF16,
                          kind="ExternalInput")
    wt = nc.dram_tensor("wt", [D, D], BF16, kind="ExternalInput")
    bindb = nc.dram_tensor("bindb", [SEG_PER_CORE, D], F32,
                           kind="ExternalInput")
    out = nc.dram_tensor("out", [SEG_PER_CORE, D], F32, kind="ExternalOutput")

    # [N_SUP, 128, K, 512]; per (t, p) the (K, 512) block is 16KB contiguous
    x_r = x.ap()[0:SUP * N_SUP, :].rearrange("(t p k) d -> t p k d",
                                             p=128, k=K)
    x_tail = x.ap()[SUP * N_SUP:RPC, :].rearrange("(p k) d -> p k d", k=KT)

    with tile.TileContext(nc) as tc:
        with tc.tile_pool(name="const", bufs=1) as const:
            iota_sb = const.tile([128, W_WIN], BF16, name="iota_sb")
            batch_sb = const.tile([128, N_PLANES], F32, name="batch_sb")
            sell_sb = const.tile([W_WIN, SEG_PER_CORE], BF16,
                                 name="sell_sb")
            wt_sb = const.tile([128, 4 * D], BF16, name="wt_sb")
            bind_sb = const.tile([SEG_PER_CORE, D], F32, name="bind_sb")
            w_sb = const.tile([W_WIN, D], BF16, name="w_sb")
            lhsT = const.tile([128, 4 * SEG_PER_CORE], BF16, name="lhsT")
            res = const.tile([SEG_PER_CORE, D], F32, name="res")

            # small inputs via SWDGE so the HWDGE x queues carry only x;
            # they land ~12us in, before the first tile's matmuls need them
            nc.gpsimd.dma_start(out=iota_sb[:, :], in_=iota[:, :])
            nc.gpsimd.dma_start(out=batch_sb[:, :], in_=batchp[:, :])
            nc.gpsimd.dma_start(out=sell_sb[:, :], in_=sell[:, :])
            for i in range(4):
                nc.gpsimd.dma_start(out=wt_sb[:, i * D:(i + 1) * D],
                                    in_=wt[i * 128:(i + 1) * 128, :])
            nc.gpsimd.dma_start(out=bind_sb[:, :], in_=bindb[:, :])

            with tc.tile_pool(name="xin", bufs=9) as xp, \
                 tc.tile_pool(name="xtail", bufs=1) as xtp, \
                 tc.tile_pool(name="ohp", bufs=16) as ohp, \
                 tc.tile_pool(name="pacc", bufs=1, space="PSUM") as pacc, \
                 tc.tile_pool(name="pepi", bufs=1, space="PSUM") as pepi:
                psW = pacc.tile([W_WIN, D], F32, name="psW")
                pt = [pepi.tile([128, SEG_PER_CORE], F32, name=f"pt{c}")
                      for c in range(4)]
                po = pepi.tile([SEG_PER_CORE, D], F32, name="po")

                def plane(oh_src_col, ps_args, xp_bf, k):
                    oh = ohp.tile([128, W_WIN], BF16, name="oh")
                    nc.vector.tensor_scalar(
                        oh[:, :], iota_sb[:, :],
                        batch_sb[:, oh_src_col:oh_src_col + 1],
                        None, mybir.AluOpType.is_equal)
                    nc.tensor.matmul(psW[:, :], oh[:, :], xp_bf[:, k, 1::2],
                                     **ps_args, skip_group_check=True)

                # queue balance: odd tiles + all of t15 on sync (16MB),
                # even tiles t0..t14 on scalar (16MB); tail 0.25MB each
                for t in range(N_SUP):
                    xt = xp.tile([128, K, D], F32, name="xt")
                    if t == N_SUP - 1:
                        # split the last full supertile for drain
                        # granularity (all chunks on sync, see above)
                        for c in range(4):
                            nc.sync.dma_start(
                                out=xt[:, 2 * c:2 * c + 2, :],
                                in_=x_r[t][:, 2 * c:2 * c + 2, :])
                    else:
                        xq = nc.scalar if t % 2 == 0 else nc.sync
                        xq.dma_start(out=xt[:, :, :], in_=x_r[t])
                    # little-endian f32: the high halfword of each element
                    # IS its truncated bf16 value -> free bf16 operand
                    xt_bf = xt[:, :, :].bitcast(BF16)
                    for k in range(K):
                        plane(t * K + k,
                              dict(start=(t == 0 and k == 0), stop=False),
                              xt_bf, k)
                # short tail supertile (the shard-length slack)
                xt2 = xtp.tile([128, KT, D], F32, name="xt2")
                nc.sync.dma_start(out=xt2[:, 0:1, :], in_=x_tail[:, 0:1, :])
                nc.scalar.dma_start(out=xt2[:, 1:2, :], in_=x_tail[:, 1:2, :])
                xt2_bf = xt2[:, :, :].bitcast(BF16)
                for k in range(KT):
                    plane(N_SUP * K + k,
                          dict(start=False, stop=(k == KT - 1)), xt2_bf, k)

                # tail: window -> owned-segment selection -> tiny linear
                nc.vector.tensor_copy(w_sb[:, :], psW[:, :])
                for c in range(4):
                    nc.tensor.matmul(pt[c][:, :],
                                     w_sb[:, c * 128:(c + 1) * 128],
                                     sell_sb[:, :], start=True, stop=True,
                                     skip_group_check=True)
                for c in range(4):
                    nc.vector.tensor_copy(
                        lhsT[:, c * SEG_PER_CORE:(c + 1) * SEG_PER_CORE],
                        pt[c][:, :])
                for c in range(4):
                    nc.tensor.matmul(
                        po[:, :],
                        lhsT[:, c * SEG_PER_CORE:(c + 1) * SEG_PER_CORE],
                        wt_sb[:, c * D:(c + 1) * D],
                        start=(c == 0), stop=(c == 3),
                        skip_group_check=True)
                # res = seg_mean @ Wt + b*(cnt>0)  (inv folded into sell)
                nc.vector.tensor_tensor(res[:, :], po[:, :], bind_sb[:, :],
                                        mybir.AluOpType.add)
                nc.sync.dma_start(out=out[:, :], in_=res[:, :])
    nc.compile()
    return nc


def make_in_maps(x, W, b, batch):
    x = np.asarray(x, dtype=np.float32)
    W = np.asarray(W, dtype=np.float32)
    b = np.asarray(b, dtype=np.float32)
    batch = np.asarray(batch).astype(np.int64)
    npbf = mybir.dt.np(BF16)
    wt = np.ascontiguousarray(W.T).astype(npbf)
    iota = np.tile(np.arange(W_WIN, dtype=np.float32), (128, 1)).astype(npbf)
    counts = np.bincount(batch, minlength=N_SEG).astype(np.float64)
    inv = (1.0 / np.maximum(counts, 1.0)).astype(np.float32)
    ind = (counts > 0).astype(np.float32)

    # segment-aligned shard bounds: core j owns segments [32j, 32j+32)
    starts = [int(np.searchsorted(batch, SEG_PER_CORE * j))
              for j in range(N_CORES)] + [len(batch)]

    in_maps = []
    for j in range(N_CORES):
        lo = SEG_PER_CORE * j
        s0, s1 = starts[j], starts[j + 1]
        L = s1 - s0
        assert L <= RPC, f"core {j}: owned rows {L} exceed {RPC}"
        # row indices, padded by repeating the last row (dummy-windowed)
        ridx = np.minimum(s0 + np.arange(RPC), len(batch) - 1)
        rel = (batch[ridx] - lo).astype(np.float32)
        rel[L:] = DUMMY  # padding rows -> discarded window slot
        assert rel[:L].min() >= 0 and rel[:L].max() < SEG_PER_CORE, \
            f"core {j}: owned rel out of range"

        bp = np.empty((128, N_PLANES), np.float32)
        planes = rel[:SUP * N_SUP].reshape(N_SUP, 128, K)
        for t in range(N_SUP):
            bp[:, t * K:(t + 1) * K] = planes[t]
        bp[:, N_SUP * K:] = rel[SUP * N_SUP:].reshape(128, KT)

        sell = np.zeros((W_WIN, SEG_PER_CORE), np.float32)
        for u in range(SEG_PER_CORE):
            sell[u, u] = inv[lo + u]
        bindb = b.reshape(1, D) * ind[lo:lo + SEG_PER_CORE, None]

        in_maps.append({
            "x": np.ascontiguousarray(x[ridx]),
            "batchp": np.ascontiguousarray(bp),
            "iota": iota,
            "sell": np.ascontiguousarray(sell.astype(npbf)),
            "wt": wt,
            "bindb": np.ascontiguousarray(bindb.astype(np.float32)),
        })
    return in_maps


_NC_CACHE = {}


def kernel(x, W, b, batch, num_segments, trace=False):
    assert int(num_segments) == N_SEG
    if "nc" not in _NC_CACHE:
        _NC_CACHE["nc"] = build_nc()
    nc = _NC_CACHE["nc"]
    in_maps = make_in_maps(x, W, b, batch)
    res = run_bass_kernel_spmd(nc, in_maps, core_ids=list(range(N_CORES)),
                               trace=trace)
    full = np.concatenate([res.results[j]["out"] for j in range(N_CORES)],
                          axis=0)
    if trace:
        return full, res
    return full
